# revision 20
# baseline (speedup 1.0000x reference)
"""Trainium2 Bass kernel for nn_AttentionBlock (GroupNorm -> 1x1 qkv conv ->
softmax attention over N=HW -> 1x1 proj -> residual).

Sharding: 8 cores = 4 images x 2 query-column halves. Each core receives its
image column-permuted so its own 2048 query columns come first; attention is
permutation-invariant over key/value positions, so k/v use all 4096 columns
in permuted order. GroupNorm stats are computed on-chip per core (full image).

Math folding done on host (tiny O(C^2) numpy):
  - gn_w folded into qkv weight columns; gn_b folded into qkv biases.
  - 1/sqrt(C) score scale folded into Wq and bq.
  - k bias dropped entirely (adds a per-row constant to scores: softmax-invariant).
  - v bias folded into proj bias (softmax rows sum to 1): bp_eff = bp + Wp @ bv.
On-chip per core:
  h = (x - mean_g) * rstd_g              (per-channel affine from group stats)
  q = Wq^T-matmul(h) + bq  (cols 0:2048) ; k = Wk-matmul(h) (all cols)
  vpos[m, c] = Wv-matmul(h)              (position-major layout)
  per 512-col tile of q:  E[m, n] = exp(k_chunk^T q_tile) accumulated flash-style:
     av[c, n] += vpos_chunk^T E ;  eacc[n] += E (DVE)
  S = ones^T eacc (all-ones 128x128 matmul -> S replicated on all partitions)
  ha = av * (1/S) ; y = x_tile + Wp-matmul(ha) + bp_eff
"""

import numpy as np

B, C, HH, WW = 4, 256, 64, 64
N = HH * WW            # 4096
NH = N // 2            # 2048 query columns per core
GROUPS = 32
GSIZE = C // GROUPS    # 8
EPS = 1e-5
NCORES = 8
P = 128
NT = NH // 512         # 4 query tiles per core
MC = N // P            # 32 key chunks
KT = N // 512          # 8 column tiles for k

_prog = None


def _build_program():
    import concourse.bacc as bacc
    import concourse.tile as tile
    from concourse import mybir

    f32 = mybir.dt.float32
    f32r = mybir.dt.float32r
    AF = mybir.ActivationFunctionType
    ALU = mybir.AluOpType

    nc = bacc.Bacc("TRN2", target_bir_lowering=False, debug=False,
                   num_devices=NCORES)

    x_d = nc.dram_tensor("x", [C, N], f32, kind="ExternalInput").ap()
    wqk_d = nc.dram_tensor("wqk", [C, 2 * C], f32r, kind="ExternalInput").ap()
    wv_d = nc.dram_tensor("wv", [C, C], f32r, kind="ExternalInput").ap()
    wp_d = nc.dram_tensor("wp", [C, C], f32r, kind="ExternalInput").ap()
    bq_d = nc.dram_tensor("bq", [C, 1], f32, kind="ExternalInput").ap()
    bp_d = nc.dram_tensor("bp", [C, 1], f32, kind="ExternalInput").ap()
    gm_d = nc.dram_tensor("gm", [P, 16], f32, kind="ExternalInput").ap()
    gt_d = nc.dram_tensor("gt", [16, P], f32, kind="ExternalInput").ap()
    onr_d = nc.dram_tensor("onr", [P, P], f32r, kind="ExternalInput").ap()
    y_d = nc.dram_tensor("y", [C, NH], f32, kind="ExternalOutput").ap()

    xv = x_d.rearrange("(j p) n -> p j n", p=P)        # [128, 2, 4096]
    wqkv = wqk_d.rearrange("(j p) o -> p j o", p=P)    # [128, 2, 512]
    wvv = wv_d.rearrange("(j p) o -> p j o", p=P)
    wpv = wp_d.rearrange("(j p) o -> p j o", p=P)
    bqv = bq_d.rearrange("(j p) o -> p j o", p=P)      # [128, 2, 1]
    bpv = bp_d.rearrange("(j p) o -> p j o", p=P)
    yv = y_d.rearrange("(j p) n -> p j n", p=P)        # [128, 2, 2048]

    with tile.TileContext(nc) as tc:
        with (
            tc.tile_pool(name="big", bufs=1) as big,
            tc.tile_pool(name="wts", bufs=1) as wts,
            tc.tile_pool(name="stats", bufs=1) as stats,
            tc.tile_pool(name="epool", bufs=6) as epool,
            tc.tile_pool(name="acc", bufs=2) as accp,
            tc.tile_pool(name="rp", bufs=2) as rp,
            tc.tile_pool(name="hap", bufs=2) as hap,
            tc.tile_pool(name="yp", bufs=2) as yp,
        ):

            # ---- load x first (critical path), 3 parallel DMA queues ----
            xs = big.tile([P, 2, N], f32)
            dma_engs = [nc.sync, nc.gpsimd, nc.scalar, nc.sync]
            for j in range(2):
                for qd in range(4):
                    sl = slice(qd * 1024, (qd + 1) * 1024)
                    dma_engs[(j * 4 + qd) % 3].dma_start(
                        out=xs[:, j, sl], in_=xv[:, j, sl])

            # ---- weights / consts (off the critical path) ----
            wqk = wts.tile([P, 2, 2 * C], f32r)
            nc.gpsimd.dma_start(out=wqk, in_=wqkv)
            wv = wts.tile([P, 2, C], f32r)
            nc.scalar.dma_start(out=wv, in_=wvv)
            wp = wts.tile([P, 2, C], f32r)
            nc.scalar.dma_start(out=wp, in_=wpv)
            bq = wts.tile([P, 2, 1], f32)
            nc.sync.dma_start(out=bq, in_=bqv)
            bp = wts.tile([P, 2, 1], f32)
            nc.sync.dma_start(out=bp, in_=bpv)
            gm = wts.tile([P, 16], f32)
            nc.sync.dma_start(out=gm, in_=gm_d)
            gt = wts.tile([16, P], f32)
            nc.sync.dma_start(out=gt, in_=gt_d)
            ones_sq = wts.tile([P, P], f32r)
            nc.sync.dma_start(out=ones_sq, in_=onr_d)
            eps_t = wts.tile([16, 1], f32)
            nc.vector.memset(eps_t, EPS)

            # ---- group stats ----
            AB = stats.tile([P, 2, 2], f32)  # per-channel (mean, rstd)
            with tc.tile_pool(name="psStat", bufs=1, space="PSUM") as psst:
                for j in range(2):
                    st6 = stats.tile([P, 8, 6], f32, tag="st6")
                    xsr = xs[:, j, :].rearrange("p (s f) -> p s f", f=512)
                    for sg in range(8):
                        nc.vector.bn_stats(out=st6[:, sg, :], in_=xsr[:, sg, :])
                    mv = stats.tile([P, 2], f32, tag="mv")
                    nc.vector.bn_aggr(out=mv, in_=st6)
                    # t2 = (mean, var + mean^2)
                    t2 = stats.tile([P, 2], f32, tag="t2")
                    nc.vector.tensor_copy(out=t2[:, 0:1], in_=mv[:, 0:1])
                    nc.vector.scalar_tensor_tensor(
                        out=t2[:, 1:2], in0=mv[:, 0:1], scalar=mv[:, 0:1],
                        in1=mv[:, 1:2], op0=ALU.mult, op1=ALU.add,
                    )
                    gagg = psst.tile([16, 2], f32, tag="gagg")
                    nc.tensor.matmul(gagg, lhsT=gm, rhs=t2, start=True, stop=True)
                    # grs = (gmean, rstd)
                    grs = stats.tile([16, 2], f32, tag="grs")
                    nc.scalar.copy(out=grs[:, 0:1], in_=gagg[:, 0:1])
                    sq = stats.tile([16, 1], f32, tag="sq")
                    nc.scalar.square(out=sq, in_=gagg[:, 0:1])
                    var = stats.tile([16, 1], f32, tag="var")
                    nc.vector.tensor_sub(out=var, in0=gagg[:, 1:2], in1=sq)
                    nc.scalar.activation(out=var, in_=var, func=AF.Sqrt,
                                         bias=eps_t, scale=1.0)
                    nc.vector.reciprocal(out=grs[:, 1:2], in_=var)
                    gb = psst.tile([P, 2], f32, tag="gb")
                    nc.tensor.matmul(gb, lhsT=gt, rhs=grs, start=True, stop=True)
                    nc.scalar.copy(out=AB[:, j, :], in_=gb)

            # ---- qkv ----
            q_s = big.tile([P, 2, NH], f32r)
            k_s = big.tile([P, 2, N], f32r)
            v_s = big.tile([P, MC, C], f32r)
            with (
                tc.tile_pool(name="hp", bufs=1) as hp,
                tc.tile_pool(name="psD", bufs=4, space="PSUM") as psd,
            ):
                hs = hp.tile([P, 2, N], f32r)
                for j in range(2):
                    for nd in range(4):
                        ns = slice(nd * 1024, (nd + 1) * 1024)
                        nc.vector.tensor_scalar(
                            out=hs[:, j, ns], in0=xs[:, j, ns],
                            scalar1=AB[:, j, 0:1], scalar2=AB[:, j, 1:2],
                            op0=ALU.subtract, op1=ALU.mult,
                        )
                # q (own half) and k (all columns)
                for jo in range(2):
                    for tt in range(NT):
                        sl = slice(tt * 512, (tt + 1) * 512)
                        ps = psd.tile([P, 512], f32, tag="mm")
                        for j in range(2):
                            nc.tensor.matmul(
                                ps, lhsT=wqk[:, j, jo * P:(jo + 1) * P],
                                rhs=hs[:, j, sl],
                                start=(j == 0), stop=(j == 1),
                            )
                        nc.vector.tensor_scalar_add(out=q_s[:, jo, sl],
                                                    in0=ps,
                                                    scalar1=bq[:, jo, :])
                for jo in range(2):
                    for tt in range(KT):
                        sl = slice(tt * 512, (tt + 1) * 512)
                        ps = psd.tile([P, 512], f32, tag="mm")
                        for j in range(2):
                            nc.tensor.matmul(
                                ps, lhsT=wqk[:, j, C + jo * P:C + (jo + 1) * P],
                                rhs=hs[:, j, sl],
                                start=(j == 0), stop=(j == 1),
                            )
                        if tt % 2 == 0:
                            nc.scalar.copy(out=k_s[:, jo, sl], in_=ps)
                        else:
                            nc.vector.tensor_copy(out=k_s[:, jo, sl], in_=ps)
                # vpos[m, c]
                for mc in range(MC):
                    msl = slice(mc * P, (mc + 1) * P)
                    ps = psd.tile([P, 512], f32, tag="mm")
                    for j in range(2):
                        nc.tensor.matmul(
                            ps[:, 0:C], lhsT=hs[:, j, msl], rhs=wv[:, j, :],
                            start=(j == 0), stop=(j == 1),
                        )
                    if mc % 2 == 0:
                        nc.scalar.copy(out=v_s[:, mc, :], in_=ps[:, 0:C])
                    else:
                        nc.vector.tensor_copy(out=v_s[:, mc, :], in_=ps[:, 0:C])

            # ---- attention ----
            with (
                tc.tile_pool(name="psQK", bufs=3, space="PSUM") as psqk,
                tc.tile_pool(name="psAV", bufs=2, space="PSUM") as psav,
                tc.tile_pool(name="psSP", bufs=1, space="PSUM") as pssp,
            ):
                for tt in range(NT):
                    sl = slice(tt * 512, (tt + 1) * 512)
                    # two interleaved exp-sum accumulators (halves the RAW chain)
                    ea = [accp.tile([P, 512], f32r, name=f"eacc{i}", tag=f"eacc{i}")
                          for i in range(2)]
                    nc.vector.memset(ea[0].bitcast(f32), 0.0)
                    nc.vector.memset(ea[1].bitcast(f32), 0.0)
                    av0 = psav.tile([P, 512], f32, tag="av0")
                    av1 = psav.tile([P, 512], f32, tag="av1")
                    # one-stage software pipeline: av[mc-1] runs while
                    # exp[mc] computes, so the PE never waits on the ACT.
                    ets = [None] * MC

                    def av_pair(mc):
                        et = ets[mc]
                        nc.tensor.matmul(av0, lhsT=v_s[:, mc, 0:P], rhs=et,
                                         start=(mc == 0), stop=(mc == MC - 1))
                        nc.tensor.matmul(av1, lhsT=v_s[:, mc, P:C], rhs=et,
                                         start=(mc == 0), stop=(mc == MC - 1))
                        acc = ea[mc % 2]
                        eng = nc.vector if mc % 2 == 0 else nc.gpsimd
                        eng.tensor_add(out=acc, in0=acc.bitcast(f32),
                                       in1=et.bitcast(f32))

                    for mc in range(MC):
                        msl = slice(mc * P, (mc + 1) * P)
                        qk = psqk.tile([P, 512], f32, tag="qk")
                        for j in range(2):
                            nc.tensor.matmul(
                                qk, lhsT=k_s[:, j, msl], rhs=q_s[:, j, sl],
                                start=(j == 0), stop=(j == 1),
                            )
                        et = epool.tile([P, 512], f32r, name=f"et{mc % 6}",
                                        tag="et")
                        ets[mc] = et
                        nc.scalar.activation(out=et, in_=qk, func=AF.Exp)
                        if mc > 0:
                            av_pair(mc - 1)
                    av_pair(MC - 1)
                    sps = pssp.tile([P, 512], f32, tag="sp")
                    nc.tensor.matmul(sps, lhsT=ones_sq, rhs=ea[0],
                                     start=True, stop=False)
                    nc.tensor.matmul(sps, lhsT=ones_sq, rhs=ea[1],
                                     start=False, stop=True)
                    rb = rp.tile([P, 512], f32, tag="rb")
                    nc.vector.reciprocal(out=rb, in_=sps)
                    ha = hap.tile([P, 2, 512], f32r, tag="ha")
                    nc.vector.tensor_mul(out=ha[:, 0, :], in0=av0, in1=rb)
                    nc.vector.tensor_mul(out=ha[:, 1, :], in0=av1, in1=rb)
                    yt = yp.tile([P, 2, 512], f32, tag="yt")
                    for jo in range(2):
                        pp = pssp.tile([P, 512], f32, tag="sp")
                        for j in range(2):
                            nc.tensor.matmul(
                                pp, lhsT=wp[:, j, jo * P:(jo + 1) * P],
                                rhs=ha[:, j, :],
                                start=(j == 0), stop=(j == 1),
                            )
                        nc.vector.scalar_tensor_tensor(
                            out=yt[:, jo, :], in0=pp, scalar=bp[:, jo, :],
                            in1=xs[:, jo, sl], op0=ALU.add, op1=ALU.add,
                        )
                    nc.sync.dma_start(out=yv[:, 0, sl], in_=yt[:, 0, :])
                    nc.sync.dma_start(out=yv[:, 1, sl], in_=yt[:, 1, :])

    nc.compile()
    return nc


def _get_prog():
    global _prog
    if _prog is None:
        _prog = _build_program()
    return _prog


def _host_prep(x, gn_w, gn_b, qkv_w, qkv_b, proj_w, proj_b):
    """Returns (shared input dict, per-core x list)."""
    x = np.asarray(x, dtype=np.float32)
    gn_w = np.asarray(gn_w, dtype=np.float32)
    gn_b = np.asarray(gn_b, dtype=np.float32)
    qkv_w = np.asarray(qkv_w, dtype=np.float32)
    qkv_b = np.asarray(qkv_b, dtype=np.float32)
    proj_w = np.asarray(proj_w, dtype=np.float32)
    proj_b = np.asarray(proj_b, dtype=np.float32)

    scale = 1.0 / np.sqrt(C).astype(np.float32)
    Wq = qkv_w[0:C] * gn_w[None, :] * scale
    bq_eff = (qkv_w[0:C] @ gn_b + qkv_b[0:C]) * scale
    Wk = qkv_w[C:2 * C] * gn_w[None, :]
    Wv = qkv_w[2 * C:3 * C] * gn_w[None, :]
    bv_eff = qkv_w[2 * C:3 * C] @ gn_b + qkv_b[2 * C:3 * C]
    bp_eff = proj_b + proj_w @ bv_eff

    wqk = np.concatenate([Wq.T, Wk.T], axis=1).astype(np.float32)  # [C, 2C]
    wv_h = np.ascontiguousarray(Wv.T, dtype=np.float32)
    wp_h = np.ascontiguousarray(proj_w.T, dtype=np.float32)

    cidx = np.arange(P)
    gm = np.zeros((P, 16), dtype=np.float32)
    gm[cidx, cidx // GSIZE] = 1.0 / GSIZE
    gt = np.zeros((16, P), dtype=np.float32)
    gt[cidx // GSIZE, cidx] = 1.0

    shared = {
        "onr": np.ones((P, P), dtype=np.float32),
        "wqk": wqk,
        "wv": wv_h,
        "wp": wp_h,
        "bq": bq_eff.reshape(C, 1).astype(np.float32),
        "bp": bp_eff.reshape(C, 1).astype(np.float32),
        "gm": gm,
        "gt": gt,
    }

    xf = x.reshape(B, C, N)
    xs_per_core = []
    for core in range(NCORES):
        b, half = core // 2, core % 2
        if half == 0:
            xc = xf[b]
        else:
            xc = np.concatenate([xf[b][:, NH:], xf[b][:, :NH]], axis=1)
        xs_per_core.append(np.ascontiguousarray(xc))
    return shared, xs_per_core


def run_sharded(inputs, trace=False, trace_kwargs=None):
    """Run the 8-core kernel. Returns (full_output, BassKernelResults)."""
    from concourse.bass_utils import run_bass_kernel_spmd

    nc = _get_prog()
    shared, xs_per_core = _host_prep(**inputs)
    in_maps = [{**shared, "x": xs_per_core[c]} for c in range(NCORES)]
    kw = {}
    if trace:
        kw["trace"] = True
        if trace_kwargs:
            kw["trace_kwargs"] = trace_kwargs
    res = run_bass_kernel_spmd(nc, in_maps, list(range(NCORES)), **kw)

    out = np.empty((B, C, N), dtype=np.float32)
    for core in range(NCORES):
        b, half = core // 2, core % 2
        yc = res.results[core]["y"]
        out[b][:, half * NH:(half + 1) * NH] = yc
    return out.reshape(B, C, HH, WW), res


def kernel(**inputs):
    out, _ = run_sharded(inputs)
    return out


# revision 21
# speedup vs baseline: 1.0169x; 1.0169x over previous
"""Trainium2 Bass kernel for nn_AttentionBlock (GroupNorm -> 1x1 qkv conv ->
softmax attention over N=HW -> 1x1 proj -> residual).

Sharding: 8 cores = 4 images x 2 query-column halves. Each core receives its
image column-permuted so its own 2048 query columns come first; attention is
permutation-invariant over key/value positions, so k/v use all 4096 columns
in permuted order. GroupNorm stats are computed on-chip per core (full image).

Math folding done on host (tiny O(C^2) numpy):
  - gn_w folded into qkv weight columns; gn_b folded into qkv biases.
  - 1/sqrt(C) score scale folded into Wq and bq.
  - k bias dropped entirely (adds a per-row constant to scores: softmax-invariant).
  - v bias folded into proj bias (softmax rows sum to 1): bp_eff = bp + Wp @ bv.
On-chip per core:
  h = (x - mean_g) * rstd_g              (per-channel affine from group stats)
  q = Wq^T-matmul(h) + bq  (cols 0:2048) ; k = Wk-matmul(h) (all cols)
  vpos[m, c] = Wv-matmul(h)              (position-major layout)
  per 512-col tile of q:  E[m, n] = exp(k_chunk^T q_tile) accumulated flash-style:
     av[c, n] += vpos_chunk^T E ;  eacc[n] += E (DVE)
  S = ones^T eacc (all-ones 128x128 matmul -> S replicated on all partitions)
  ha = av * (1/S) ; y = x_tile + Wp-matmul(ha) + bp_eff
"""

import numpy as np

B, C, HH, WW = 4, 256, 64, 64
N = HH * WW            # 4096
NH = N // 2            # 2048 query columns per core
GROUPS = 32
GSIZE = C // GROUPS    # 8
EPS = 1e-5
NCORES = 8
P = 128
NT = NH // 512         # 4 query tiles per core
MC = N // P            # 32 key chunks
KT = N // 512          # 8 column tiles for k

_prog = None


def _build_program():
    import concourse.bacc as bacc
    import concourse.tile as tile
    from concourse import mybir

    f32 = mybir.dt.float32
    f32r = mybir.dt.float32r
    AF = mybir.ActivationFunctionType
    ALU = mybir.AluOpType

    nc = bacc.Bacc("TRN2", target_bir_lowering=False, debug=False,
                   num_devices=NCORES)

    x_d = nc.dram_tensor("x", [C, N], f32, kind="ExternalInput").ap()
    wqk_d = nc.dram_tensor("wqk", [C, 2 * C], f32r, kind="ExternalInput").ap()
    wv_d = nc.dram_tensor("wv", [C, C], f32r, kind="ExternalInput").ap()
    wp_d = nc.dram_tensor("wp", [C, C], f32r, kind="ExternalInput").ap()
    bq_d = nc.dram_tensor("bq", [C, 1], f32, kind="ExternalInput").ap()
    bp_d = nc.dram_tensor("bp", [C, 1], f32, kind="ExternalInput").ap()
    gm_d = nc.dram_tensor("gm", [P, 16], f32, kind="ExternalInput").ap()
    gt_d = nc.dram_tensor("gt", [16, P], f32, kind="ExternalInput").ap()
    onr_d = nc.dram_tensor("onr", [P, P], f32r, kind="ExternalInput").ap()
    y_d = nc.dram_tensor("y", [C, NH], f32, kind="ExternalOutput").ap()

    xv = x_d.rearrange("(j p) n -> p j n", p=P)        # [128, 2, 4096]
    wqkv = wqk_d.rearrange("(j p) o -> p j o", p=P)    # [128, 2, 512]
    wvv = wv_d.rearrange("(j p) o -> p j o", p=P)
    wpv = wp_d.rearrange("(j p) o -> p j o", p=P)
    bqv = bq_d.rearrange("(j p) o -> p j o", p=P)      # [128, 2, 1]
    bpv = bp_d.rearrange("(j p) o -> p j o", p=P)
    yv = y_d.rearrange("(j p) n -> p j n", p=P)        # [128, 2, 2048]

    with tile.TileContext(nc) as tc:
        with (
            tc.tile_pool(name="big", bufs=1) as big,
            tc.tile_pool(name="wts", bufs=1) as wts,
            tc.tile_pool(name="stats", bufs=1) as stats,
            tc.tile_pool(name="epool", bufs=6) as epool,
            tc.tile_pool(name="acc", bufs=2) as accp,
            tc.tile_pool(name="rp", bufs=2) as rp,
            tc.tile_pool(name="hap", bufs=2) as hap,
            tc.tile_pool(name="yp", bufs=2) as yp,
        ):

            # PE warmup: dense dummy matmuls fill the x-DMA wait so the HAM
            # clock gate opens (K=8/8) before the real matmul stream starts.
            dummy = wts.tile([P, 512], f32)
            nc.vector.memset(dummy, 0.0)
            with tc.tile_pool(name="psW", bufs=1, space="PSUM") as psw:
                wps = psw.tile([P, 512], f32, tag="w")
                dr = dummy.bitcast(f32r)
                for _ in range(45):
                    nc.tensor.matmul(wps, lhsT=dr[:, 0:P], rhs=dr,
                                     start=True, stop=True)

            # ---- load x first (critical path), 3 parallel DMA queues ----
            xs = big.tile([P, 2, N], f32)
            dma_engs = [nc.sync, nc.gpsimd, nc.scalar, nc.sync]
            for j in range(2):
                for qd in range(4):
                    sl = slice(qd * 1024, (qd + 1) * 1024)
                    dma_engs[(j * 4 + qd) % 3].dma_start(
                        out=xs[:, j, sl], in_=xv[:, j, sl])

            # ---- weights / consts (off the critical path) ----
            wqk = wts.tile([P, 2, 2 * C], f32r)
            nc.gpsimd.dma_start(out=wqk, in_=wqkv)
            wv = wts.tile([P, 2, C], f32r)
            nc.scalar.dma_start(out=wv, in_=wvv)
            wp = wts.tile([P, 2, C], f32r)
            nc.scalar.dma_start(out=wp, in_=wpv)
            bq = wts.tile([P, 2, 1], f32)
            nc.sync.dma_start(out=bq, in_=bqv)
            bp = wts.tile([P, 2, 1], f32)
            nc.sync.dma_start(out=bp, in_=bpv)
            gm = wts.tile([P, 16], f32)
            nc.sync.dma_start(out=gm, in_=gm_d)
            gt = wts.tile([16, P], f32)
            nc.sync.dma_start(out=gt, in_=gt_d)
            ones_sq = wts.tile([P, P], f32r)
            nc.sync.dma_start(out=ones_sq, in_=onr_d)
            eps_t = wts.tile([16, 1], f32)
            nc.vector.memset(eps_t, EPS)

            # ---- group stats ----
            AB = stats.tile([P, 2, 2], f32)  # per-channel (mean, rstd)
            with tc.tile_pool(name="psStat", bufs=1, space="PSUM") as psst:
                for j in range(2):
                    st6 = stats.tile([P, 8, 6], f32, tag="st6")
                    xsr = xs[:, j, :].rearrange("p (s f) -> p s f", f=512)
                    for sg in range(8):
                        nc.vector.bn_stats(out=st6[:, sg, :], in_=xsr[:, sg, :])
                    mv = stats.tile([P, 2], f32, tag="mv")
                    nc.vector.bn_aggr(out=mv, in_=st6)
                    # t2 = (mean, var + mean^2)
                    t2 = stats.tile([P, 2], f32, tag="t2")
                    nc.vector.tensor_copy(out=t2[:, 0:1], in_=mv[:, 0:1])
                    nc.vector.scalar_tensor_tensor(
                        out=t2[:, 1:2], in0=mv[:, 0:1], scalar=mv[:, 0:1],
                        in1=mv[:, 1:2], op0=ALU.mult, op1=ALU.add,
                    )
                    gagg = psst.tile([16, 2], f32, tag="gagg")
                    nc.tensor.matmul(gagg, lhsT=gm, rhs=t2, start=True, stop=True)
                    # grs = (gmean, rstd)
                    grs = stats.tile([16, 2], f32, tag="grs")
                    nc.scalar.copy(out=grs[:, 0:1], in_=gagg[:, 0:1])
                    sq = stats.tile([16, 1], f32, tag="sq")
                    nc.scalar.square(out=sq, in_=gagg[:, 0:1])
                    var = stats.tile([16, 1], f32, tag="var")
                    nc.vector.tensor_sub(out=var, in0=gagg[:, 1:2], in1=sq)
                    nc.scalar.activation(out=var, in_=var, func=AF.Sqrt,
                                         bias=eps_t, scale=1.0)
                    nc.vector.reciprocal(out=grs[:, 1:2], in_=var)
                    gb = psst.tile([P, 2], f32, tag="gb")
                    nc.tensor.matmul(gb, lhsT=gt, rhs=grs, start=True, stop=True)
                    nc.scalar.copy(out=AB[:, j, :], in_=gb)

            # ---- qkv ----
            q_s = big.tile([P, 2, NH], f32r)
            k_s = big.tile([P, 2, N], f32r)
            v_s = big.tile([P, MC, C], f32r)
            with (
                tc.tile_pool(name="hp", bufs=1) as hp,
                tc.tile_pool(name="psD", bufs=4, space="PSUM") as psd,
            ):
                hs = hp.tile([P, 2, N], f32r)
                for j in range(2):
                    for nd in range(4):
                        ns = slice(nd * 1024, (nd + 1) * 1024)
                        nc.vector.tensor_scalar(
                            out=hs[:, j, ns], in0=xs[:, j, ns],
                            scalar1=AB[:, j, 0:1], scalar2=AB[:, j, 1:2],
                            op0=ALU.subtract, op1=ALU.mult,
                        )
                # q (own half) and k (all columns)
                for jo in range(2):
                    for tt in range(NT):
                        sl = slice(tt * 512, (tt + 1) * 512)
                        ps = psd.tile([P, 512], f32, tag="mm")
                        for j in range(2):
                            nc.tensor.matmul(
                                ps, lhsT=wqk[:, j, jo * P:(jo + 1) * P],
                                rhs=hs[:, j, sl],
                                start=(j == 0), stop=(j == 1),
                            )
                        nc.vector.tensor_scalar_add(out=q_s[:, jo, sl],
                                                    in0=ps,
                                                    scalar1=bq[:, jo, :])
                for jo in range(2):
                    for tt in range(KT):
                        sl = slice(tt * 512, (tt + 1) * 512)
                        ps = psd.tile([P, 512], f32, tag="mm")
                        for j in range(2):
                            nc.tensor.matmul(
                                ps, lhsT=wqk[:, j, C + jo * P:C + (jo + 1) * P],
                                rhs=hs[:, j, sl],
                                start=(j == 0), stop=(j == 1),
                            )
                        if tt % 2 == 0:
                            nc.scalar.copy(out=k_s[:, jo, sl], in_=ps)
                        else:
                            nc.vector.tensor_copy(out=k_s[:, jo, sl], in_=ps)
                # vpos[m, c]
                for mc in range(MC):
                    msl = slice(mc * P, (mc + 1) * P)
                    ps = psd.tile([P, 512], f32, tag="mm")
                    for j in range(2):
                        nc.tensor.matmul(
                            ps[:, 0:C], lhsT=hs[:, j, msl], rhs=wv[:, j, :],
                            start=(j == 0), stop=(j == 1),
                        )
                    if mc % 2 == 0:
                        nc.scalar.copy(out=v_s[:, mc, :], in_=ps[:, 0:C])
                    else:
                        nc.vector.tensor_copy(out=v_s[:, mc, :], in_=ps[:, 0:C])

            # ---- attention ----
            with (
                tc.tile_pool(name="psQK", bufs=3, space="PSUM") as psqk,
                tc.tile_pool(name="psAV", bufs=2, space="PSUM") as psav,
                tc.tile_pool(name="psSP", bufs=1, space="PSUM") as pssp,
            ):
                for tt in range(NT):
                    sl = slice(tt * 512, (tt + 1) * 512)
                    # two interleaved exp-sum accumulators (halves the RAW chain)
                    ea = [accp.tile([P, 512], f32r, name=f"eacc{i}", tag=f"eacc{i}")
                          for i in range(2)]
                    nc.vector.memset(ea[0].bitcast(f32), 0.0)
                    nc.vector.memset(ea[1].bitcast(f32), 0.0)
                    av0 = psav.tile([P, 512], f32, tag="av0")
                    av1 = psav.tile([P, 512], f32, tag="av1")
                    # one-stage software pipeline: av[mc-1] runs while
                    # exp[mc] computes, so the PE never waits on the ACT.
                    ets = [None] * MC

                    def av_pair(mc):
                        et = ets[mc]
                        nc.tensor.matmul(av0, lhsT=v_s[:, mc, 0:P], rhs=et,
                                         start=(mc == 0), stop=(mc == MC - 1))
                        nc.tensor.matmul(av1, lhsT=v_s[:, mc, P:C], rhs=et,
                                         start=(mc == 0), stop=(mc == MC - 1))
                        acc = ea[mc % 2]
                        nc.vector.tensor_add(out=acc, in0=acc.bitcast(f32),
                                             in1=et.bitcast(f32))

                    for mc in range(MC):
                        msl = slice(mc * P, (mc + 1) * P)
                        qk = psqk.tile([P, 512], f32, tag="qk")
                        for j in range(2):
                            nc.tensor.matmul(
                                qk, lhsT=k_s[:, j, msl], rhs=q_s[:, j, sl],
                                start=(j == 0), stop=(j == 1),
                            )
                        et = epool.tile([P, 512], f32r, name=f"et{mc % 6}",
                                        tag="et")
                        ets[mc] = et
                        nc.scalar.activation(out=et, in_=qk, func=AF.Exp)
                        if mc > 0:
                            av_pair(mc - 1)
                    av_pair(MC - 1)
                    sps = pssp.tile([P, 512], f32, tag="sp")
                    nc.tensor.matmul(sps, lhsT=ones_sq, rhs=ea[0],
                                     start=True, stop=False)
                    nc.tensor.matmul(sps, lhsT=ones_sq, rhs=ea[1],
                                     start=False, stop=True)
                    rb = rp.tile([P, 512], f32, tag="rb")
                    nc.vector.reciprocal(out=rb, in_=sps)
                    ha = hap.tile([P, 2, 512], f32r, tag="ha")
                    nc.vector.tensor_mul(out=ha[:, 0, :], in0=av0, in1=rb)
                    nc.vector.tensor_mul(out=ha[:, 1, :], in0=av1, in1=rb)
                    yt = yp.tile([P, 2, 512], f32, tag="yt")
                    for jo in range(2):
                        pp = pssp.tile([P, 512], f32, tag="sp")
                        for j in range(2):
                            nc.tensor.matmul(
                                pp, lhsT=wp[:, j, jo * P:(jo + 1) * P],
                                rhs=ha[:, j, :],
                                start=(j == 0), stop=(j == 1),
                            )
                        nc.vector.scalar_tensor_tensor(
                            out=yt[:, jo, :], in0=pp, scalar=bp[:, jo, :],
                            in1=xs[:, jo, sl], op0=ALU.add, op1=ALU.add,
                        )
                    nc.sync.dma_start(out=yv[:, :, sl], in_=yt)

    nc.compile()
    return nc


def _get_prog():
    global _prog
    if _prog is None:
        _prog = _build_program()
    return _prog


def _host_prep(x, gn_w, gn_b, qkv_w, qkv_b, proj_w, proj_b):
    """Returns (shared input dict, per-core x list)."""
    x = np.asarray(x, dtype=np.float32)
    gn_w = np.asarray(gn_w, dtype=np.float32)
    gn_b = np.asarray(gn_b, dtype=np.float32)
    qkv_w = np.asarray(qkv_w, dtype=np.float32)
    qkv_b = np.asarray(qkv_b, dtype=np.float32)
    proj_w = np.asarray(proj_w, dtype=np.float32)
    proj_b = np.asarray(proj_b, dtype=np.float32)

    scale = 1.0 / np.sqrt(C).astype(np.float32)
    Wq = qkv_w[0:C] * gn_w[None, :] * scale
    bq_eff = (qkv_w[0:C] @ gn_b + qkv_b[0:C]) * scale
    Wk = qkv_w[C:2 * C] * gn_w[None, :]
    Wv = qkv_w[2 * C:3 * C] * gn_w[None, :]
    bv_eff = qkv_w[2 * C:3 * C] @ gn_b + qkv_b[2 * C:3 * C]
    bp_eff = proj_b + proj_w @ bv_eff

    wqk = np.concatenate([Wq.T, Wk.T], axis=1).astype(np.float32)  # [C, 2C]
    wv_h = np.ascontiguousarray(Wv.T, dtype=np.float32)
    wp_h = np.ascontiguousarray(proj_w.T, dtype=np.float32)

    cidx = np.arange(P)
    gm = np.zeros((P, 16), dtype=np.float32)
    gm[cidx, cidx // GSIZE] = 1.0 / GSIZE
    gt = np.zeros((16, P), dtype=np.float32)
    gt[cidx // GSIZE, cidx] = 1.0

    shared = {
        "onr": np.ones((P, P), dtype=np.float32),
        "wqk": wqk,
        "wv": wv_h,
        "wp": wp_h,
        "bq": bq_eff.reshape(C, 1).astype(np.float32),
        "bp": bp_eff.reshape(C, 1).astype(np.float32),
        "gm": gm,
        "gt": gt,
    }

    xf = x.reshape(B, C, N)
    xs_per_core = []
    for core in range(NCORES):
        b, half = core // 2, core % 2
        if half == 0:
            xc = xf[b]
        else:
            xc = np.concatenate([xf[b][:, NH:], xf[b][:, :NH]], axis=1)
        xs_per_core.append(np.ascontiguousarray(xc))
    return shared, xs_per_core


def run_sharded(inputs, trace=False, trace_kwargs=None):
    """Run the 8-core kernel. Returns (full_output, BassKernelResults)."""
    from concourse.bass_utils import run_bass_kernel_spmd

    nc = _get_prog()
    shared, xs_per_core = _host_prep(**inputs)
    in_maps = [{**shared, "x": xs_per_core[c]} for c in range(NCORES)]
    kw = {}
    if trace:
        kw["trace"] = True
        if trace_kwargs:
            kw["trace_kwargs"] = trace_kwargs
    res = run_bass_kernel_spmd(nc, in_maps, list(range(NCORES)), **kw)

    out = np.empty((B, C, N), dtype=np.float32)
    for core in range(NCORES):
        b, half = core // 2, core % 2
        yc = res.results[core]["y"]
        out[b][:, half * NH:(half + 1) * NH] = yc
    return out.reshape(B, C, HH, WW), res


def kernel(**inputs):
    out, _ = run_sharded(inputs)
    return out
